# revision 7
# baseline (speedup 1.0000x reference)
"""2-layer multi-edge-type GAT on Trainium2, 8-core SPMD.

Strategy: shard edges by dst owner (core c owns dst rows [c*R, (c+1)*R) of
each layer). Host-side (int-only) prep sorts each core's edges by dst,
buckets them into 32-dst windows, and pads per-window edge lists to 128-edge
chunks with chunk counts uniform across cores (SPMD: one program).

Device per core:
  A) z0 table build: z0 = feat @ W0 row tiles via PE (lhsT = host-transposed
     featT), fused el/er columns, written to HBM tables [z|el] + [er].
  B) Edge aggregation (per edge type): indirect-DMA gather of [z|el] rows by
     src and er rows by dst; ex = exp(leakyrelu(el+er)) on ACT; one-hot
     window mask via is_equal on DVE; per 128-edge chunk one matmul
     psum[32 dst, :] += OH.T @ [ex*z | ex] accumulated per window in PSUM;
     per-window softmax normalization (the denominator comes from the ex
     columns; segment-max subtraction is dropped — softmax is shift
     invariant and the logits are O(1), fp32-safe).
  C) h = relu(mean_h(gat_a + gat_b)); z1/el1/er1 rows computed shard-wise,
     AllGathered into full tables.
  D) Same edge aggregation for layer 1; out = norm_a + norm_b + 2*b1.
"""

import math
import sys

import numpy as np

if "/opt/trn_rl_repo" not in sys.path:
    sys.path.insert(0, "/opt/trn_rl_repo")

import concourse.bacc as bacc
import concourse.bass as bass
import concourse.mybir as mybir
import concourse.tile as tile
from concourse.bass_utils import run_bass_kernel_spmd
from concourse.masks import make_identity

F32 = mybir.dt.float32
I32 = mybir.dt.int32
AF = mybir.ActivationFunctionType
ALU = mybir.AluOpType

P = 128
NEG_SLOPE = 0.2
SEG_EPS = 1e-9


# ----------------------------------------------------------------------------
# host-side (integer-only) edge preprocessing
# ----------------------------------------------------------------------------

def cdiv(a, b):
    return (a + b - 1) // b


def prep_edges(src, dst, R, win, n_cores, remap_src, remap_dst):
    """Partition edges by dst-owning core, sort by dst, bucket into windows.

    Chunk counts per window are maxed across cores so all cores share one
    program. Padding slots get src/dst table row 0 and col -1 (masked out).

    Returns (cpw, per_core) where cpw is [n_win] chunks-per-window and
    per_core[c] = dict(srcT, dstT, colT) each [128, C] (chunk-major cols).
    """
    R_pad = cdiv(R, win) * win
    n_win = R_pad // win
    cores = []
    for c in range(n_cores):
        lo = c * R
        m = (dst >= lo) & (dst < lo + R)
        s = src[m].astype(np.int64)
        d_glob = dst[m].astype(np.int64)
        d = d_glob - lo
        o = np.argsort(d, kind="stable")
        s, d, d_glob = s[o], d[o], d_glob[o]
        cnt = np.bincount(d // win, minlength=n_win)
        cores.append((s, d, d_glob, cnt))
    counts = np.stack([c[3] for c in cores])
    cpw = np.maximum(cdiv(counts.max(axis=0), P), 1).astype(np.int64)
    C = int(cpw.sum())
    slot0 = np.concatenate([[0], np.cumsum(cpw)[:-1]]) * P
    per_core = []
    for c in range(n_cores):
        s, d, d_glob, cnt = cores[c]
        src_a = np.zeros(C * P, np.int64)
        dst_a = np.zeros(C * P, np.int64)
        col_a = np.full(C * P, -1.0, np.float32)
        w = d // win
        woff = np.concatenate([[0], np.cumsum(cnt)[:-1]])
        pos = slot0[w] + (np.arange(len(d)) - woff[w])
        src_a[pos] = remap_src(s)
        dst_a[pos] = remap_dst(d_glob)
        col_a[pos] = (d % win).astype(np.float32)
        per_core.append(dict(
            srcT=np.ascontiguousarray(src_a.reshape(C, P).T).astype(np.int32),
            dstT=np.ascontiguousarray(dst_a.reshape(C, P).T).astype(np.int32),
            colT=np.ascontiguousarray(col_a.reshape(C, P).T).astype(np.float32),
        ))
    return cpw, per_core


def block_diag_attn(attn):
    """[H, D] -> [H*D, H] block-diagonal selection matrix (no arithmetic)."""
    H, D = attn.shape
    out = np.zeros((H * D, H), np.float32)
    for h in range(H):
        out[h * D:(h + 1) * D, h] = attn[h]
    return out


# ----------------------------------------------------------------------------
# device program
# ----------------------------------------------------------------------------

class Cfg:
    def __init__(self, n_cores=8, N0=50000, N1=20000, N2=10000,
                 F_in=64, H0=4, D0=32, H1=1, D1=32, win=32, G=32):
        self.n_cores = n_cores
        self.N0, self.N1, self.N2 = N0, N1, N2
        self.F_in, self.H0, self.D0, self.H1, self.D1 = F_in, H0, D0, H1, D1
        self.win, self.G = win, G
        self.R0 = N1 // n_cores
        self.R1 = N2 // n_cores
        self.R0_pad = cdiv(self.R0, win) * win
        self.R1_pad = cdiv(self.R1, win) * win
        self.n_win0 = self.R0_pad // win
        self.n_win1 = self.R1_pad // win
        self.N0_pad = cdiv(N0, P) * P
        self.NH = n_cores * self.R0_pad          # h_full / z1 table rows
        self.T0 = H0 * D0 + H0                   # z0 row: [z | el]
        self.T1 = H1 * D1 + H1                   # z1 row: [z | el]
        self.RW0 = H0 * D0 + H0                  # matmul rhs width: [ex*z | ex]
        self.RW1 = H1 * D1 + H1

    def remap1(self, i):
        """h-node id (0..N1) -> z1 table row (pad-interleaved shards)."""
        return (i // self.R0) * self.R0_pad + (i % self.R0)


def chunk_windows(cpw):
    """Per-chunk window id + first/last flags from chunks-per-window."""
    win_of, first, last = [], [], []
    for w, k in enumerate(cpw):
        for j in range(int(k)):
            win_of.append(w)
            first.append(j == 0)
            last.append(j == int(k) - 1)
    return win_of, first, last


def build_program(cfg, cpw0a, cpw0b, cpw1a, cpw1b):
    n_cores = cfg.n_cores
    nc = bacc.Bacc("TRN2", target_bir_lowering=False, debug=False,
                   num_devices=n_cores)

    # ---- external inputs --------------------------------------------------
    featT = nc.dram_tensor("featT", [cfg.F_in, cfg.N0_pad], F32, kind="ExternalInput")
    W0 = nc.dram_tensor("W0", [cfg.F_in, cfg.H0 * cfg.D0], F32, kind="ExternalInput")
    Aler0 = nc.dram_tensor("Aler0", [cfg.H0 * cfg.D0, 2 * cfg.H0], F32, kind="ExternalInput")
    b0r = nc.dram_tensor("b0r", [cfg.H0, cfg.D0], F32, kind="ExternalInput")
    W1 = nc.dram_tensor("W1", [cfg.D0, cfg.H1 * cfg.D1], F32, kind="ExternalInput")
    Aler1 = nc.dram_tensor("Aler1", [cfg.H1 * cfg.D1, 2 * cfg.H1], F32, kind="ExternalInput")
    b1r = nc.dram_tensor("b1r", [1, cfg.D1], F32, kind="ExternalInput")
    iota = nc.dram_tensor("iota", [P, cfg.win], F32, kind="ExternalInput")
    ones4 = nc.dram_tensor("ones4", [cfg.H0, 1], F32, kind="ExternalInput")
    twos = nc.dram_tensor("twos", [1, P], F32, kind="ExternalInput")

    edge_in = {}
    for name, cpw in (("e0a", cpw0a), ("e0b", cpw0b), ("e1a", cpw1a), ("e1b", cpw1b)):
        C = int(cpw.sum())
        edge_in[name] = dict(
            src=nc.dram_tensor(f"{name}_src", [P, C], I32, kind="ExternalInput"),
            dst=nc.dram_tensor(f"{name}_dst", [P, C], I32, kind="ExternalInput"),
            col=nc.dram_tensor(f"{name}_col", [P, C], F32, kind="ExternalInput"),
            cpw=cpw, C=C,
        )

    out_t = nc.dram_tensor("out", [cfg.R1_pad, cfg.D1], F32, kind="ExternalOutput")

    with tile.TileContext(nc) as tc:
        from contextlib import ExitStack
        with ExitStack() as ctx:
            const = ctx.enter_context(tc.tile_pool(name="const", bufs=1))
            sbuf = ctx.enter_context(tc.tile_pool(name="sbuf", bufs=3))
            big = ctx.enter_context(tc.tile_pool(name="big", bufs=2))
            stage = ctx.enter_context(tc.tile_pool(name="stage", bufs=2))
            small = ctx.enter_context(tc.tile_pool(name="small", bufs=3))
            psum = ctx.enter_context(tc.tile_pool(name="psum", bufs=2, space="PSUM"))
            psw = ctx.enter_context(tc.tile_pool(name="psw", bufs=4, space="PSUM"))
            dram = ctx.enter_context(tc.tile_pool(name="dram", bufs=1, space="DRAM"))

            # ---- internal DRAM ------------------------------------------
            z0_table = dram.tile([cfg.N0_pad, cfg.T0], F32)
            er0_table = dram.tile([cfg.N0_pad, cfg.H0], F32)
            h_stage_a = dram.tile([cfg.R0_pad, cfg.H0 * cfg.D0], F32)
            h_stage_b = dram.tile([cfg.R0_pad, cfg.H0 * cfg.D0], F32)
            z1_shard = dram.tile([cfg.R0_pad, cfg.T1], F32)
            er1_shard = dram.tile([cfg.R0_pad, cfg.H1], F32)
            z1_table = dram.tile([cfg.NH, cfg.T1], F32, addr_space="Shared")
            er1_table = dram.tile([cfg.NH, cfg.H1], F32, addr_space="Shared")
            o_stage_a = dram.tile([cfg.R1_pad, cfg.D1], F32)
            o_stage_b = dram.tile([cfg.R1_pad, cfg.D1], F32)

            # ---- constants to SBUF --------------------------------------
            ident = const.tile([P, P], F32)
            make_identity(nc, ident[:])
            W0_sb = const.tile([cfg.F_in, cfg.H0 * cfg.D0], F32)
            nc.sync.dma_start(out=W0_sb[:], in_=W0[:])
            Aler0_sb = const.tile([cfg.H0 * cfg.D0, 2 * cfg.H0], F32)
            nc.sync.dma_start(out=Aler0_sb[:], in_=Aler0[:])
            b0_sb = const.tile([cfg.H0, cfg.D0], F32)
            nc.sync.dma_start(out=b0_sb[:], in_=b0r[:])
            W1_sb = const.tile([cfg.D0, cfg.H1 * cfg.D1], F32)
            nc.sync.dma_start(out=W1_sb[:], in_=W1[:])
            Aler1_sb = const.tile([cfg.H1 * cfg.D1, 2 * cfg.H1], F32)
            nc.sync.dma_start(out=Aler1_sb[:], in_=Aler1[:])
            b1_sb = const.tile([1, cfg.D1], F32)
            nc.sync.dma_start(out=b1_sb[:], in_=b1r[:])
            iota_sb = const.tile([P, cfg.win], F32)
            nc.sync.dma_start(out=iota_sb[:], in_=iota[:])
            ones4_sb = const.tile([cfg.H0, 1], F32)
            nc.sync.dma_start(out=ones4_sb[:], in_=ones4[:])
            twos_sb = const.tile([1, P], F32)
            nc.sync.dma_start(out=twos_sb[:], in_=twos[:])

            F_in, HD0 = cfg.F_in, cfg.H0 * cfg.D0

            # W0e = [W0 | W0 @ Al0 | W0 @ Ar0]  [F_in, HD0 + 2*H0]
            pt = psum.tile([HD0, F_in], F32, tag="ps_m")
            nc.tensor.transpose(out=pt[:], in_=W0_sb[:], identity=ident[:F_in, :F_in])
            W0T_sb = const.tile([HD0, F_in], F32)
            nc.scalar.copy(out=W0T_sb[:], in_=pt[:])
            pe = psum.tile([F_in, 2 * cfg.H0], F32, tag="ps_m")
            nc.tensor.matmul(out=pe[:], lhsT=W0T_sb[:], rhs=Aler0_sb[:], start=True, stop=True)
            W0e_sb = const.tile([F_in, HD0 + 2 * cfg.H0], F32)
            nc.vector.tensor_copy(out=W0e_sb[:, :HD0], in_=W0_sb[:])
            nc.vector.tensor_copy(out=W0e_sb[:, HD0:], in_=pe[:])

            # W1e = [W1 | W1 @ Al1 | W1 @ Ar1]  [D0, H1*D1 + 2*H1]
            HD1 = cfg.H1 * cfg.D1
            pt1 = psum.tile([HD1, cfg.D0], F32, tag="ps_m")
            nc.tensor.transpose(out=pt1[:], in_=W1_sb[:], identity=ident[:cfg.D0, :cfg.D0])
            W1T_sb = const.tile([HD1, cfg.D0], F32)
            nc.scalar.copy(out=W1T_sb[:], in_=pt1[:])
            pe1 = psum.tile([cfg.D0, 2 * cfg.H1], F32, tag="ps_m")
            nc.tensor.matmul(out=pe1[:], lhsT=W1T_sb[:], rhs=Aler1_sb[:], start=True, stop=True)
            W1e_sb = const.tile([cfg.D0, HD1 + 2 * cfg.H1], F32)
            nc.vector.tensor_copy(out=W1e_sb[:, :HD1], in_=W1_sb[:])
            nc.vector.tensor_copy(out=W1e_sb[:, HD1:], in_=pe1[:])

            # bias broadcast tiles: mb2 [P, D0] = 2*sum_h b0 ; bias1 [P, D1] = 2*b1
            ps_s = psum.tile([1, cfg.D0], F32, tag="ps_m")
            nc.tensor.matmul(out=ps_s[:], lhsT=ones4_sb[:], rhs=b0_sb[:], start=True, stop=True)
            sb0_sb = const.tile([1, cfg.D0], F32)
            nc.scalar.copy(out=sb0_sb[:], in_=ps_s[:])
            ps_mb = psum.tile([P, cfg.D0], F32, tag="ps_m")
            nc.tensor.matmul(out=ps_mb[:], lhsT=twos_sb[:], rhs=sb0_sb[:], start=True, stop=True)
            mb2_sb = const.tile([P, cfg.D0], F32)
            nc.scalar.copy(out=mb2_sb[:], in_=ps_mb[:])
            ps_b1 = psum.tile([P, cfg.D1], F32, tag="ps_m")
            nc.tensor.matmul(out=ps_b1[:], lhsT=twos_sb[:], rhs=b1_sb[:], start=True, stop=True)
            bias1_sb = const.tile([P, cfg.D1], F32)
            nc.scalar.copy(out=bias1_sb[:], in_=ps_b1[:])

            # ---- Phase A: z0 / er0 tables -------------------------------
            ZB = 8
            n_t0 = cfg.N0_pad // P
            for bi in range(0, n_t0, ZB):
                bsz = min(ZB, n_t0 - bi)
                ft = sbuf.tile([F_in, bsz * P], F32, tag="ft")
                nc.sync.dma_start(out=ft[:], in_=featT[:, bi * P:(bi + bsz) * P])
                st = stage.tile([P, bsz * cfg.T0], F32, tag="zst")
                ste = stage.tile([P, bsz * cfg.H0], F32, tag="zste")
                for j in range(bsz):
                    zp = psum.tile([P, HD0 + 2 * cfg.H0], F32, tag="ps_m")
                    nc.tensor.matmul(out=zp[:], lhsT=ft[:, j * P:(j + 1) * P],
                                     rhs=W0e_sb[:], start=True, stop=True)
                    nc.scalar.copy(out=st[:, j * cfg.T0:(j + 1) * cfg.T0],
                                   in_=zp[:, :cfg.T0])
                    nc.scalar.copy(out=ste[:, j * cfg.H0:(j + 1) * cfg.H0],
                                   in_=zp[:, cfg.T0:])
                nc.sync.dma_start(
                    out=z0_table[bi * P:(bi + bsz) * P, :].rearrange(
                        "(j p) w -> p j w", p=P),
                    in_=st[:].rearrange("p (j w) -> p j w", w=cfg.T0))
                nc.sync.dma_start(
                    out=er0_table[bi * P:(bi + bsz) * P, :].rearrange(
                        "(j p) w -> p j w", p=P),
                    in_=ste[:].rearrange("p (j w) -> p j w", w=cfg.H0))

            # ---- shared edge-aggregation phase --------------------------
            def edge_phase(name, H, D, table, er_table, ed, stage_dram):
                HD = H * D
                rw = HD + H
                cpw = ed["cpw"]
                Cn = ed["C"]
                win_of, first, last = chunk_windows(cpw)
                G = cfg.G
                psumw = {}
                SB = 8  # windows per h-stage store batch
                stg = {"t": None}
                n_win = len(cpw)

                def flush_stage(w_hi):
                    # store windows [w_lo, w_hi] of current batch
                    w_lo = (w_hi // SB) * SB
                    k = w_hi - w_lo + 1
                    nc.sync.dma_start(
                        out=stage_dram[w_lo * cfg.win:(w_hi + 1) * cfg.win, :]
                        .rearrange("(j d) f -> d j f", d=cfg.win),
                        in_=stg["t"][:, :k * HD].rearrange("d (j f) -> d j f", f=HD))
                    stg["t"] = None

                for g0 in range(0, Cn, G):
                    gsz = min(G, Cn - g0)
                    rows = big.tile([P, G * rw], F32, tag=f"rows_{H}")
                    srco = sbuf.tile([P, G], I32, tag="srco")
                    dsto = sbuf.tile([P, G], I32, tag="dsto")
                    colt = sbuf.tile([P, G], F32, tag="colt")
                    nc.sync.dma_start(out=srco[:, :gsz], in_=ed["src"][:, g0:g0 + gsz])
                    nc.sync.dma_start(out=dsto[:, :gsz], in_=ed["dst"][:, g0:g0 + gsz])
                    nc.sync.dma_start(out=colt[:, :gsz], in_=ed["col"][:, g0:g0 + gsz])
                    rows3 = rows[:].rearrange("p (j w) -> p j w", w=rw)
                    er = sbuf.tile([P, G * H], F32, tag=f"er_{H}")
                    er3 = er[:].rearrange("p (j h) -> p j h", h=H)
                    for j in range(gsz):
                        nc.gpsimd.indirect_dma_start(
                            out=rows3[:, j, :HD + H], out_offset=None,
                            in_=table[:],
                            in_offset=bass.IndirectOffsetOnAxis(
                                ap=srco[:, j:j + 1], axis=0))
                        nc.gpsimd.indirect_dma_start(
                            out=er3[:, j, :], out_offset=None,
                            in_=er_table[:],
                            in_offset=bass.IndirectOffsetOnAxis(
                                ap=dsto[:, j:j + 1], axis=0))
                    # ex = exp(lrelu(el + er)) written into rows[:, :, HD:HD+H]
                    nc.vector.tensor_tensor(
                        out=rows3[:, :gsz, HD:HD + H],
                        in0=rows3[:, :gsz, HD:HD + H], in1=er3[:, :gsz, :], op=ALU.add)
                    lk = sbuf.tile([P, G * H], F32, tag=f"lk_{H}")
                    lk3 = lk[:].rearrange("p (j h) -> p j h", h=H)
                    nc.vector.tensor_scalar(
                        out=lk3[:, :gsz], in0=rows3[:, :gsz, HD:HD + H],
                        scalar1=NEG_SLOPE, scalar2=None, op0=ALU.mult)
                    nc.vector.tensor_tensor(
                        out=rows3[:, :gsz, HD:HD + H],
                        in0=rows3[:, :gsz, HD:HD + H], in1=lk3[:, :gsz],
                        op=ALU.max)
                    nc.scalar.activation(
                        out=rows3[:, :gsz, HD:HD + H], in_=rows3[:, :gsz, HD:HD + H],
                        func=AF.Exp)
                    # zx: scale z block by per-head ex (in place on rows)
                    r4 = rows3[:, :gsz, :HD].rearrange(
                        "p j (h d) -> p j h d", h=H)
                    ex4 = rows3[:, :gsz, HD:HD + H].unsqueeze(3).to_broadcast(
                        [P, gsz, H, D])
                    nc.vector.tensor_tensor(
                        out=r4, in0=r4, in1=ex4, op=ALU.mult)
                    # one-hot window mask
                    cmp = big.tile([P, G * cfg.win], F32, tag="cmp")
                    cmp3 = cmp[:].rearrange("p (j w) -> p j w", w=cfg.win)
                    nc.vector.tensor_tensor(
                        out=cmp3[:, :gsz],
                        in0=colt[:, :gsz].unsqueeze(2).to_broadcast(
                            [P, gsz, cfg.win]),
                        in1=iota_sb[:].unsqueeze(1).to_broadcast(
                            [P, gsz, cfg.win]),
                        op=ALU.is_equal)
                    for j in range(gsz):
                        ch = g0 + j
                        w = win_of[ch]
                        if first[ch]:
                            psumw[w] = psw.tile([cfg.win, rw], F32, tag="ps_w", name=f"pw_{name}_{w}")
                        nc.tensor.matmul(
                            out=psumw[w][:], lhsT=cmp3[:, j, :], rhs=rows3[:, j, :],
                            start=first[ch], stop=last[ch])
                        if last[ch]:
                            pw = psumw.pop(w)
                            sm = small.tile([cfg.win, H], F32, tag="sm")
                            nc.vector.tensor_scalar(
                                out=sm[:], in0=pw[:, HD:HD + H], scalar1=SEG_EPS,
                                scalar2=None, op0=ALU.max)
                            rs = small.tile([cfg.win, H], F32, tag="rs")
                            nc.vector.reciprocal(out=rs[:], in_=sm[:])
                            if stg["t"] is None:
                                stg["t"] = stage.tile([cfg.win, SB * HD], F32, tag="hstg", name=f"stg_{name}_{w}")
                            slot = w % SB
                            dstv = stg["t"][:, slot * HD:(slot + 1) * HD] \
                                .rearrange("d (h f) -> d h f", h=H)
                            nc.vector.tensor_tensor(
                                out=dstv,
                                in0=pw[:, :HD].rearrange("d (h f) -> d h f", h=H),
                                in1=rs[:].unsqueeze(2).to_broadcast(
                                    [cfg.win, H, D]),
                                op=ALU.mult)
                            if slot == SB - 1 or w == n_win - 1:
                                flush_stage(w)

            # ---- Phase B: layer-0 edges ---------------------------------
            edge_phase("e0a", cfg.H0, cfg.D0, z0_table, er0_table,
                       edge_in["e0a"], h_stage_a)
            edge_phase("e0b", cfg.H0, cfg.D0, z0_table, er0_table,
                       edge_in["e0b"], h_stage_b)

            # ---- Phase C: h build + z1 shard + AllGather ----------------
            n_ht = cdiv(cfg.R0_pad, P)
            for i in range(n_ht):
                p = min(P, cfg.R0_pad - i * P)
                at = sbuf.tile([P, HD0], F32, tag="ha")
                bt = sbuf.tile([P, HD0], F32, tag="hb")
                nc.sync.dma_start(out=at[:p], in_=h_stage_a[i * P:i * P + p, :])
                nc.sync.dma_start(out=bt[:p], in_=h_stage_b[i * P:i * P + p, :])
                nc.vector.tensor_add(out=at[:p], in0=at[:p], in1=bt[:p])
                hs = sbuf.tile([P, cfg.D0], F32, tag="hs")
                nc.vector.tensor_add(out=hs[:p], in0=at[:p, 0:cfg.D0],
                                     in1=at[:p, cfg.D0:2 * cfg.D0])
                for h in range(2, cfg.H0):
                    nc.vector.tensor_add(
                        out=hs[:p], in0=hs[:p],
                        in1=at[:p, h * cfg.D0:(h + 1) * cfg.D0])
                nc.vector.tensor_add(out=hs[:p], in0=hs[:p], in1=mb2_sb[:p])
                hr = sbuf.tile([P, cfg.D0], F32, tag="hr")
                nc.scalar.activation(out=hr[:p], in_=hs[:p], func=AF.Relu,
                                     scale=1.0 / cfg.H0)
                htp = psum.tile([cfg.D0, P], F32, tag="ps_m")
                nc.tensor.transpose(out=htp[:, :p], in_=hr[:p],
                                    identity=ident[:p, :p])
                hts = sbuf.tile([cfg.D0, P], F32, tag="hts")
                nc.scalar.copy(out=hts[:, :p], in_=htp[:, :p])
                zp1 = psum.tile([P, HD1 + 2 * cfg.H1], F32, tag="ps_m")
                nc.tensor.matmul(out=zp1[:p], lhsT=hts[:, :p], rhs=W1e_sb[:],
                                 start=True, stop=True)
                z1s = sbuf.tile([P, cfg.T1], F32, tag="z1s")
                e1s = sbuf.tile([P, cfg.H1], F32, tag="e1s")
                nc.scalar.copy(out=z1s[:p], in_=zp1[:p, :cfg.T1])
                nc.scalar.copy(out=e1s[:p], in_=zp1[:p, cfg.T1:])
                nc.sync.dma_start(out=z1_shard[i * P:i * P + p, :], in_=z1s[:p])
                nc.sync.dma_start(out=er1_shard[i * P:i * P + p, :], in_=e1s[:p])

            nc.gpsimd.collective_compute(
                "AllGather", ALU.bypass,
                replica_groups=[list(range(n_cores))],
                ins=[z1_shard.opt()], outs=[z1_table.opt()])
            nc.gpsimd.collective_compute(
                "AllGather", ALU.bypass,
                replica_groups=[list(range(n_cores))],
                ins=[er1_shard.opt()], outs=[er1_table.opt()])

            # ---- Phase D: layer-1 edges ---------------------------------
            edge_phase("e1a", cfg.H1, cfg.D1, z1_table, er1_table,
                       edge_in["e1a"], o_stage_a)
            edge_phase("e1b", cfg.H1, cfg.D1, z1_table, er1_table,
                       edge_in["e1b"], o_stage_b)

            # ---- final combine ------------------------------------------
            n_ot = cdiv(cfg.R1_pad, P)
            for i in range(n_ot):
                p = min(P, cfg.R1_pad - i * P)
                oa = sbuf.tile([P, cfg.D1], F32, tag="oa")
                ob = sbuf.tile([P, cfg.D1], F32, tag="ob")
                nc.sync.dma_start(out=oa[:p], in_=o_stage_a[i * P:i * P + p, :])
                nc.sync.dma_start(out=ob[:p], in_=o_stage_b[i * P:i * P + p, :])
                nc.vector.tensor_add(out=oa[:p], in0=oa[:p], in1=ob[:p])
                nc.vector.tensor_add(out=oa[:p], in0=oa[:p], in1=bias1_sb[:p])
                nc.sync.dma_start(out=out_t[i * P:i * P + p, :], in_=oa[:p])

    nc.compile()
    return nc


# ----------------------------------------------------------------------------
# host driver
# ----------------------------------------------------------------------------

def make_host_inputs(cfg, feat, src0a, dst0a, src0b, dst0b, src1a, dst1a,
                     src1b, dst1b, W0, attn_l0, attn_r0, b0, W1, attn_l1,
                     attn_r1, b1):
    feat = np.asarray(feat, np.float32)
    featT = np.zeros((cfg.F_in, cfg.N0_pad), np.float32)
    featT[:, :cfg.N0] = feat.T

    ident0 = lambda i: i
    remap1 = cfg.remap1
    cpw0a, pc0a = prep_edges(np.asarray(src0a), np.asarray(dst0a), cfg.R0,
                             cfg.win, cfg.n_cores, ident0, ident0)
    cpw0b, pc0b = prep_edges(np.asarray(src0b), np.asarray(dst0b), cfg.R0,
                             cfg.win, cfg.n_cores, ident0, ident0)
    cpw1a, pc1a = prep_edges(np.asarray(src1a), np.asarray(dst1a), cfg.R1,
                             cfg.win, cfg.n_cores, remap1, remap1)
    cpw1b, pc1b = prep_edges(np.asarray(src1b), np.asarray(dst1b), cfg.R1,
                             cfg.win, cfg.n_cores, remap1, remap1)

    shared = dict(
        featT=featT,
        W0=np.asarray(W0, np.float32),
        Aler0=np.concatenate([block_diag_attn(np.asarray(attn_l0, np.float32)),
                              block_diag_attn(np.asarray(attn_r0, np.float32))],
                             axis=1),
        b0r=np.asarray(b0, np.float32).reshape(cfg.H0, cfg.D0),
        W1=np.asarray(W1, np.float32),
        Aler1=np.concatenate([block_diag_attn(np.asarray(attn_l1, np.float32)),
                              block_diag_attn(np.asarray(attn_r1, np.float32))],
                             axis=1),
        b1r=np.asarray(b1, np.float32).reshape(1, cfg.D1),
        iota=np.tile(np.arange(cfg.win, dtype=np.float32), (P, 1)),
        ones4=np.ones((cfg.H0, 1), np.float32),
        twos=np.full((1, P), 2.0, np.float32),
    )
    in_maps = []
    for c in range(cfg.n_cores):
        m = dict(shared)
        for name, pc in (("e0a", pc0a), ("e0b", pc0b), ("e1a", pc1a),
                         ("e1b", pc1b)):
            m[f"{name}_src"] = pc[c]["srcT"]
            m[f"{name}_dst"] = pc[c]["dstT"]
            m[f"{name}_col"] = pc[c]["colT"]
        in_maps.append(m)
    return in_maps, (cpw0a, cpw0b, cpw1a, cpw1b)


def assemble_output(cfg, results):
    outs = [results[c]["out"][:cfg.R1, :] for c in range(cfg.n_cores)]
    out = np.concatenate(outs, axis=0)
    return out.reshape(cfg.N2, cfg.H1, cfg.D1).astype(np.float32)


_CACHED = {}


def kernel(**inputs):
    cfg = Cfg()
    in_maps, cpws = make_host_inputs(cfg, **inputs)
    key = tuple(tuple(c) for c in cpws)
    if key not in _CACHED:
        _CACHED[key] = build_program(cfg, *cpws)
    nc = _CACHED[key]
    res = run_bass_kernel_spmd(nc, in_maps, list(range(cfg.n_cores)))
    return assemble_output(cfg, res.results)


# revision 11
# speedup vs baseline: 1.5613x; 1.5613x over previous
"""2-layer multi-edge-type GAT on Trainium2, 8-core SPMD.

Strategy: shard edges by dst owner (core c owns dst rows [c*R, (c+1)*R) of
each layer). Host-side (int-only) prep sorts each core's edges by dst,
buckets them into 32-dst windows, and pads per-window edge lists to 128-edge
chunks with chunk counts uniform across cores (SPMD: one program).

Device per core:
  A) z0 table build: z0 = feat @ W0 row tiles via PE (lhsT = host-transposed
     featT), fused el/er columns, written to HBM tables [z|el] + [er].
  B) Edge aggregation (per edge type): indirect-DMA gather of [z|el] rows by
     src and er rows by dst; ex = exp(leakyrelu(el+er)) on ACT; one-hot
     window mask via is_equal on DVE; per 128-edge chunk one matmul
     psum[32 dst, :] += OH.T @ [ex*z | ex] accumulated per window in PSUM;
     per-window softmax normalization (the denominator comes from the ex
     columns; segment-max subtraction is dropped — softmax is shift
     invariant and the logits are O(1), fp32-safe).
  C) h = relu(mean_h(gat_a + gat_b)); z1/el1/er1 rows computed shard-wise,
     AllGathered into full tables.
  D) Same edge aggregation for layer 1; out = norm_a + norm_b + 2*b1.
"""

import math
import sys

import numpy as np

if "/opt/trn_rl_repo" not in sys.path:
    sys.path.insert(0, "/opt/trn_rl_repo")

import concourse.bacc as bacc
import concourse.bass as bass
import concourse.mybir as mybir
import concourse.tile as tile
from concourse.bass_utils import run_bass_kernel_spmd
from concourse.masks import make_identity

F32 = mybir.dt.float32
I32 = mybir.dt.int32
AF = mybir.ActivationFunctionType
ALU = mybir.AluOpType

P = 128
NEG_SLOPE = 0.2
SEG_EPS = 1e-9


# ----------------------------------------------------------------------------
# host-side (integer-only) edge preprocessing
# ----------------------------------------------------------------------------

def cdiv(a, b):
    return (a + b - 1) // b


def prep_edges(src, dst, R, win, n_cores, remap_src, remap_dst):
    """Partition edges by dst-owning core, sort by dst, bucket into windows.

    Chunk counts per window are maxed across cores so all cores share one
    program. Padding slots get src/dst table row 0 and col -1 (masked out).

    Returns (cpw, per_core) where cpw is [n_win] chunks-per-window and
    per_core[c] = dict(srcT, dstT, colT) each [128, C] (chunk-major cols).
    """
    R_pad = cdiv(R, win) * win
    n_win = R_pad // win
    cores = []
    for c in range(n_cores):
        lo = c * R
        m = (dst >= lo) & (dst < lo + R)
        s = src[m].astype(np.int64)
        d_glob = dst[m].astype(np.int64)
        d = d_glob - lo
        o = np.argsort(d, kind="stable")
        s, d, d_glob = s[o], d[o], d_glob[o]
        cnt = np.bincount(d // win, minlength=n_win)
        cores.append((s, d, d_glob, cnt))
    counts = np.stack([c[3] for c in cores])
    cpw = np.maximum(cdiv(counts.max(axis=0), P), 1).astype(np.int64)
    C = int(cpw.sum())
    slot0 = np.concatenate([[0], np.cumsum(cpw)[:-1]]) * P
    per_core = []
    for c in range(n_cores):
        s, d, d_glob, cnt = cores[c]
        src_a = np.zeros(C * P, np.int64)
        dst_a = np.zeros(C * P, np.int64)
        col_a = np.full(C * P, -1.0, np.float32)
        w = d // win
        woff = np.concatenate([[0], np.cumsum(cnt)[:-1]])
        pos = slot0[w] + (np.arange(len(d)) - woff[w])
        src_a[pos] = remap_src(s)
        dst_a[pos] = remap_dst(d_glob)
        col_a[pos] = (d % win).astype(np.float32)
        per_core.append(dict(
            srcT=np.ascontiguousarray(src_a.reshape(C, P).T).astype(np.int32),
            dstT=np.ascontiguousarray(dst_a.reshape(C, P).T).astype(np.int32),
            colT=np.ascontiguousarray(col_a.reshape(C, P).T).astype(np.float32),
        ))
    return cpw, per_core


def block_diag_attn(attn):
    """[H, D] -> [H*D, H] block-diagonal selection matrix (no arithmetic)."""
    H, D = attn.shape
    out = np.zeros((H * D, H), np.float32)
    for h in range(H):
        out[h * D:(h + 1) * D, h] = attn[h]
    return out


# ----------------------------------------------------------------------------
# device program
# ----------------------------------------------------------------------------

class Cfg:
    def __init__(self, n_cores=8, N0=50000, N1=20000, N2=10000,
                 F_in=64, H0=4, D0=32, H1=1, D1=32, win=32, G=32):
        self.n_cores = n_cores
        self.N0, self.N1, self.N2 = N0, N1, N2
        self.F_in, self.H0, self.D0, self.H1, self.D1 = F_in, H0, D0, H1, D1
        self.win, self.G = win, G
        self.R0 = N1 // n_cores
        self.R1 = N2 // n_cores
        self.R0_pad = cdiv(self.R0, win) * win
        self.R1_pad = cdiv(self.R1, win) * win
        self.n_win0 = self.R0_pad // win
        self.n_win1 = self.R1_pad // win
        self.N0_pad = cdiv(N0, P) * P
        self.NH = n_cores * self.R0_pad          # h_full / z1 table rows
        self.T0 = H0 * D0 + H0                   # z0 row: [z | el]
        self.T1 = H1 * D1 + H1                   # z1 row: [z | el]
        self.RW0 = H0 * D0 + H0                  # matmul rhs width: [ex*z | ex]
        self.RW1 = H1 * D1 + H1

    def remap1(self, i):
        """h-node id (0..N1) -> z1 table row (pad-interleaved shards)."""
        return (i // self.R0) * self.R0_pad + (i % self.R0)


def chunk_windows(cpw):
    """Per-chunk window id + first/last flags from chunks-per-window."""
    win_of, first, last = [], [], []
    for w, k in enumerate(cpw):
        for j in range(int(k)):
            win_of.append(w)
            first.append(j == 0)
            last.append(j == int(k) - 1)
    return win_of, first, last


def build_program(cfg, cpw0a, cpw0b, cpw1a, cpw1b):
    n_cores = cfg.n_cores
    nc = bacc.Bacc("TRN2", target_bir_lowering=False, debug=False,
                   num_devices=n_cores)

    # ---- external inputs --------------------------------------------------
    featT = nc.dram_tensor("featT", [cfg.F_in, cfg.N0_pad], F32, kind="ExternalInput")
    W0 = nc.dram_tensor("W0", [cfg.F_in, cfg.H0 * cfg.D0], F32, kind="ExternalInput")
    Aler0 = nc.dram_tensor("Aler0", [cfg.H0 * cfg.D0, 2 * cfg.H0], F32, kind="ExternalInput")
    b0r = nc.dram_tensor("b0r", [cfg.H0, cfg.D0], F32, kind="ExternalInput")
    W1 = nc.dram_tensor("W1", [cfg.D0, cfg.H1 * cfg.D1], F32, kind="ExternalInput")
    Aler1 = nc.dram_tensor("Aler1", [cfg.H1 * cfg.D1, 2 * cfg.H1], F32, kind="ExternalInput")
    b1r = nc.dram_tensor("b1r", [1, cfg.D1], F32, kind="ExternalInput")
    iota = nc.dram_tensor("iota", [P, cfg.win], F32, kind="ExternalInput")
    ones4 = nc.dram_tensor("ones4", [cfg.H0, 1], F32, kind="ExternalInput")
    twos = nc.dram_tensor("twos", [1, P], F32, kind="ExternalInput")
    n_er0 = cdiv(cfg.R0_pad, P)
    n_er1 = cdiv(cfg.R1_pad, P)
    er0rows = nc.dram_tensor("er0rows", [P, n_er0], I32, kind="ExternalInput")
    er1rows = nc.dram_tensor("er1rows", [P, n_er1], I32, kind="ExternalInput")

    edge_in = {}
    for name, cpw in (("e0a", cpw0a), ("e0b", cpw0b), ("e1a", cpw1a), ("e1b", cpw1b)):
        C = int(cpw.sum())
        edge_in[name] = dict(
            src=nc.dram_tensor(f"{name}_src", [P, C], I32, kind="ExternalInput"),
            col=nc.dram_tensor(f"{name}_col", [P, C], F32, kind="ExternalInput"),
            cpw=cpw, C=C,
        )

    out_t = nc.dram_tensor("out", [cfg.R1_pad, cfg.D1], F32, kind="ExternalOutput")

    with tile.TileContext(nc) as tc:
        from contextlib import ExitStack
        with ExitStack() as ctx:
            const = ctx.enter_context(tc.tile_pool(name="const", bufs=1))
            sbuf = ctx.enter_context(tc.tile_pool(name="sbuf", bufs=3))
            big = ctx.enter_context(tc.tile_pool(name="big", bufs=2))
            stage = ctx.enter_context(tc.tile_pool(name="stage", bufs=2))
            small = ctx.enter_context(tc.tile_pool(name="small", bufs=3))
            psum = ctx.enter_context(tc.tile_pool(name="psum", bufs=2, space="PSUM"))
            psw = ctx.enter_context(tc.tile_pool(name="psw", bufs=3, space="PSUM"))
            pse = ctx.enter_context(tc.tile_pool(name="pse", bufs=3, space="PSUM"))
            dram = ctx.enter_context(tc.tile_pool(name="dram", bufs=1, space="DRAM"))

            # ---- internal DRAM ------------------------------------------
            z0_table = dram.tile([cfg.N0_pad, cfg.T0], F32)
            er0_table = dram.tile([cfg.N0_pad, cfg.H0], F32)
            h_stage_a = dram.tile([cfg.R0_pad, cfg.H0 * cfg.D0], F32)
            h_stage_b = dram.tile([cfg.R0_pad, cfg.H0 * cfg.D0], F32)
            z1_shard = dram.tile([cfg.R0_pad, cfg.T1], F32)
            er1_shard = dram.tile([cfg.R0_pad, cfg.H1], F32)
            z1_table = dram.tile([cfg.NH, cfg.T1], F32, addr_space="Shared")
            er1_table = dram.tile([cfg.NH, cfg.H1], F32, addr_space="Shared")
            o_stage_a = dram.tile([cfg.R1_pad, cfg.D1], F32)
            o_stage_b = dram.tile([cfg.R1_pad, cfg.D1], F32)

            # ---- constants to SBUF --------------------------------------
            ident = const.tile([P, P], F32)
            make_identity(nc, ident[:])
            W0_sb = const.tile([cfg.F_in, cfg.H0 * cfg.D0], F32)
            nc.sync.dma_start(out=W0_sb[:], in_=W0[:])
            Aler0_sb = const.tile([cfg.H0 * cfg.D0, 2 * cfg.H0], F32)
            nc.sync.dma_start(out=Aler0_sb[:], in_=Aler0[:])
            b0_sb = const.tile([cfg.H0, cfg.D0], F32)
            nc.sync.dma_start(out=b0_sb[:], in_=b0r[:])
            W1_sb = const.tile([cfg.D0, cfg.H1 * cfg.D1], F32)
            nc.sync.dma_start(out=W1_sb[:], in_=W1[:])
            Aler1_sb = const.tile([cfg.H1 * cfg.D1, 2 * cfg.H1], F32)
            nc.sync.dma_start(out=Aler1_sb[:], in_=Aler1[:])
            b1_sb = const.tile([1, cfg.D1], F32)
            nc.sync.dma_start(out=b1_sb[:], in_=b1r[:])
            iota_sb = const.tile([P, cfg.win], F32)
            nc.sync.dma_start(out=iota_sb[:], in_=iota[:])
            ones4_sb = const.tile([cfg.H0, 1], F32)
            nc.sync.dma_start(out=ones4_sb[:], in_=ones4[:])
            twos_sb = const.tile([1, P], F32)
            nc.sync.dma_start(out=twos_sb[:], in_=twos[:])

            F_in, HD0 = cfg.F_in, cfg.H0 * cfg.D0

            # W0e = [W0 | W0 @ Al0 | W0 @ Ar0]  [F_in, HD0 + 2*H0]
            pt = psum.tile([HD0, F_in], F32, tag="ps_m")
            nc.tensor.transpose(out=pt[:], in_=W0_sb[:], identity=ident[:F_in, :F_in])
            W0T_sb = const.tile([HD0, F_in], F32)
            nc.scalar.copy(out=W0T_sb[:], in_=pt[:])
            pe = psum.tile([F_in, 2 * cfg.H0], F32, tag="ps_m")
            nc.tensor.matmul(out=pe[:], lhsT=W0T_sb[:], rhs=Aler0_sb[:], start=True, stop=True)
            W0e_sb = const.tile([F_in, HD0 + 2 * cfg.H0], F32)
            nc.vector.tensor_copy(out=W0e_sb[:, :HD0], in_=W0_sb[:])
            nc.vector.tensor_copy(out=W0e_sb[:, HD0:], in_=pe[:])

            # W1e = [W1 | W1 @ Al1 | W1 @ Ar1]  [D0, H1*D1 + 2*H1]
            HD1 = cfg.H1 * cfg.D1
            pt1 = psum.tile([HD1, cfg.D0], F32, tag="ps_m")
            nc.tensor.transpose(out=pt1[:], in_=W1_sb[:], identity=ident[:cfg.D0, :cfg.D0])
            W1T_sb = const.tile([HD1, cfg.D0], F32)
            nc.scalar.copy(out=W1T_sb[:], in_=pt1[:])
            pe1 = psum.tile([cfg.D0, 2 * cfg.H1], F32, tag="ps_m")
            nc.tensor.matmul(out=pe1[:], lhsT=W1T_sb[:], rhs=Aler1_sb[:], start=True, stop=True)
            W1e_sb = const.tile([cfg.D0, HD1 + 2 * cfg.H1], F32)
            nc.vector.tensor_copy(out=W1e_sb[:, :HD1], in_=W1_sb[:])
            nc.vector.tensor_copy(out=W1e_sb[:, HD1:], in_=pe1[:])

            # bias broadcast tiles: mb2 [P, D0] = 2*sum_h b0 ; bias1 [P, D1] = 2*b1
            ps_s = psum.tile([1, cfg.D0], F32, tag="ps_m")
            nc.tensor.matmul(out=ps_s[:], lhsT=ones4_sb[:], rhs=b0_sb[:], start=True, stop=True)
            sb0_sb = const.tile([1, cfg.D0], F32)
            nc.scalar.copy(out=sb0_sb[:], in_=ps_s[:])
            ps_mb = psum.tile([P, cfg.D0], F32, tag="ps_m")
            nc.tensor.matmul(out=ps_mb[:], lhsT=twos_sb[:], rhs=sb0_sb[:], start=True, stop=True)
            mb2_sb = const.tile([P, cfg.D0], F32)
            nc.scalar.copy(out=mb2_sb[:], in_=ps_mb[:])
            ps_b1 = psum.tile([P, cfg.D1], F32, tag="ps_m")
            nc.tensor.matmul(out=ps_b1[:], lhsT=twos_sb[:], rhs=b1_sb[:], start=True, stop=True)
            bias1_sb = const.tile([P, cfg.D1], F32)
            nc.scalar.copy(out=bias1_sb[:], in_=ps_b1[:])

            # ---- Phase A: z0 / er0 tables -------------------------------
            ZB = 8
            n_t0 = cfg.N0_pad // P
            for bi in range(0, n_t0, ZB):
                bsz = min(ZB, n_t0 - bi)
                ft = sbuf.tile([F_in, bsz * P], F32, tag="ft")
                nc.sync.dma_start(out=ft[:], in_=featT[:, bi * P:(bi + bsz) * P])
                st = stage.tile([P, bsz * cfg.T0], F32, tag="zst")
                ste = stage.tile([P, bsz * cfg.H0], F32, tag="zste")
                for j in range(bsz):
                    zp = psum.tile([P, HD0 + 2 * cfg.H0], F32, tag="ps_m")
                    nc.tensor.matmul(out=zp[:], lhsT=ft[:, j * P:(j + 1) * P],
                                     rhs=W0e_sb[:], start=True, stop=True)
                    nc.scalar.copy(out=st[:, j * cfg.T0:(j + 1) * cfg.T0],
                                   in_=zp[:, :cfg.T0])
                    nc.scalar.copy(out=ste[:, j * cfg.H0:(j + 1) * cfg.H0],
                                   in_=zp[:, cfg.T0:])
                nc.sync.dma_start(
                    out=z0_table[bi * P:(bi + bsz) * P, :].rearrange(
                        "(j p) w -> p j w", p=P),
                    in_=st[:].rearrange("p (j w) -> p j w", w=cfg.T0))
                nc.sync.dma_start(
                    out=er0_table[bi * P:(bi + bsz) * P, :].rearrange(
                        "(j p) w -> p j w", p=P),
                    in_=ste[:].rearrange("p (j w) -> p j w", w=cfg.H0))

            # ---- er window tables: gather core-local er rows once -------
            def load_er_local(er_table, rows_in, n_blk, H, nm):
                t = const.tile([P, n_blk * H], F32, name=f"erloc_{nm}")
                ro = const.tile([P, n_blk], I32, name=f"errow_{nm}")
                nc.sync.dma_start(out=ro[:], in_=rows_in[:])
                for k in range(n_blk):
                    nc.gpsimd.indirect_dma_start(
                        out=t[:, k * H:(k + 1) * H], out_offset=None,
                        in_=er_table[:],
                        in_offset=bass.IndirectOffsetOnAxis(ap=ro[:, k:k + 1],
                                                            axis=0))
                # reshuffle [128, n_blk*H] -> [32, (4*n_blk)*H]: window-major
                # columns so every er_win slice starts at partition 0.
                tf = const.tile([32, n_blk * 4 * H], F32, name=f"erflat_{nm}")
                tf3 = tf[:].rearrange("d (q r h) -> d q r h", r=4, h=H)
                for r in range(4):
                    nc.sync.dma_start(
                        out=tf3[:, :, r, :],
                        in_=t[r * 32:(r + 1) * 32, :].rearrange(
                            "d (q h) -> d q h", h=H))
                return tf

            # ---- shared edge-aggregation phase --------------------------
            def edge_phase(name, H, D, table, er_loc, ed, stage_dram):
                HD = H * D
                rw = HD + H
                cpw = ed["cpw"]
                Cn = ed["C"]
                win_of, first, last = chunk_windows(cpw)
                G = cfg.G
                psumw = {}
                SB = 8  # windows per h-stage store batch
                stg = {"t": None}
                n_win = len(cpw)

                def flush_stage(w_hi):
                    # store windows [w_lo, w_hi] of current batch
                    w_lo = (w_hi // SB) * SB
                    k = w_hi - w_lo + 1
                    nc.sync.dma_start(
                        out=stage_dram[w_lo * cfg.win:(w_hi + 1) * cfg.win, :]
                        .rearrange("(j d) f -> d j f", d=cfg.win),
                        in_=stg["t"][:, :k * HD].rearrange("d (j f) -> d j f", f=HD))
                    stg["t"] = None

                for g0 in range(0, Cn, G):
                    gsz = min(G, Cn - g0)
                    rows = big.tile([P, G * rw], F32, tag=f"rows_{H}")
                    srco = sbuf.tile([P, G], I32, tag="srco")
                    colt = sbuf.tile([P, G], F32, tag="colt")
                    nc.sync.dma_start(out=srco[:, :gsz], in_=ed["src"][:, g0:g0 + gsz])
                    nc.sync.dma_start(out=colt[:, :gsz], in_=ed["col"][:, g0:g0 + gsz])
                    rows3 = rows[:].rearrange("p (j w) -> p j w", w=rw)
                    for j in range(gsz):
                        nc.gpsimd.indirect_dma_start(
                            out=rows3[:, j, :HD + H], out_offset=None,
                            in_=table[:],
                            in_offset=bass.IndirectOffsetOnAxis(
                                ap=srco[:, j:j + 1], axis=0))
                    # one-hot window mask (built before er so cmpT can source er)
                    cmp = big.tile([P, G * cfg.win], F32, tag="cmp")
                    cmp3 = cmp[:].rearrange("p (j w) -> p j w", w=cfg.win)
                    nc.vector.tensor_tensor(
                        out=cmp3[:, :gsz],
                        in0=colt[:, :gsz].unsqueeze(2).to_broadcast(
                            [P, gsz, cfg.win]),
                        in1=iota_sb[:].unsqueeze(1).to_broadcast(
                            [P, gsz, cfg.win]),
                        op=ALU.is_equal)
                    # er per edge: er_edge = cmpT.T @ er_win  (PE, no gather)
                    for j in range(gsz):
                        ch = g0 + j
                        w = win_of[ch]
                        ctp = pse.tile([cfg.win, P], F32, tag="ps_e",
                                       name=f"ctp_{name}_{ch}")
                        nc.tensor.transpose(out=ctp[:], in_=cmp3[:, j, :],
                                            identity=ident[:])
                        cts = sbuf.tile([cfg.win, P], F32, tag="cts")
                        nc.scalar.copy(out=cts[:], in_=ctp[:])
                        erp = pse.tile([P, H], F32, tag="ps_e",
                                       name=f"erp_{name}_{ch}")
                        nc.tensor.matmul(
                            out=erp[:], lhsT=cts[:],
                            rhs=er_loc[:, w * H:(w + 1) * H],
                            start=True, stop=True)
                        nc.vector.tensor_tensor(
                            out=rows3[:, j, HD:HD + H],
                            in0=rows3[:, j, HD:HD + H], in1=erp[:], op=ALU.add)
                    lk = sbuf.tile([P, G * H], F32, tag=f"lk_{H}")
                    lk3 = lk[:].rearrange("p (j h) -> p j h", h=H)
                    nc.vector.tensor_scalar(
                        out=lk3[:, :gsz], in0=rows3[:, :gsz, HD:HD + H],
                        scalar1=NEG_SLOPE, scalar2=None, op0=ALU.mult)
                    nc.vector.tensor_tensor(
                        out=rows3[:, :gsz, HD:HD + H],
                        in0=rows3[:, :gsz, HD:HD + H], in1=lk3[:, :gsz],
                        op=ALU.max)
                    nc.scalar.activation(
                        out=rows3[:, :gsz, HD:HD + H], in_=rows3[:, :gsz, HD:HD + H],
                        func=AF.Exp)
                    # zx: scale z block by per-head ex (in place on rows)
                    r4 = rows3[:, :gsz, :HD].rearrange(
                        "p j (h d) -> p j h d", h=H)
                    ex4 = rows3[:, :gsz, HD:HD + H].unsqueeze(3).to_broadcast(
                        [P, gsz, H, D])
                    nc.vector.tensor_tensor(
                        out=r4, in0=r4, in1=ex4, op=ALU.mult)
                    for j in range(gsz):
                        ch = g0 + j
                        w = win_of[ch]
                        if first[ch]:
                            psumw[w] = psw.tile([cfg.win, rw], F32, tag="ps_w", name=f"pw_{name}_{w}")
                        nc.tensor.matmul(
                            out=psumw[w][:], lhsT=cmp3[:, j, :], rhs=rows3[:, j, :],
                            start=first[ch], stop=last[ch])
                        if last[ch]:
                            pw = psumw.pop(w)
                            sm = small.tile([cfg.win, H], F32, tag="sm")
                            nc.vector.tensor_scalar(
                                out=sm[:], in0=pw[:, HD:HD + H], scalar1=SEG_EPS,
                                scalar2=None, op0=ALU.max)
                            rs = small.tile([cfg.win, H], F32, tag="rs")
                            nc.vector.reciprocal(out=rs[:], in_=sm[:])
                            if stg["t"] is None:
                                stg["t"] = stage.tile([cfg.win, SB * HD], F32, tag="hstg", name=f"stg_{name}_{w}")
                            slot = w % SB
                            dstv = stg["t"][:, slot * HD:(slot + 1) * HD] \
                                .rearrange("d (h f) -> d h f", h=H)
                            nc.vector.tensor_tensor(
                                out=dstv,
                                in0=pw[:, :HD].rearrange("d (h f) -> d h f", h=H),
                                in1=rs[:].unsqueeze(2).to_broadcast(
                                    [cfg.win, H, D]),
                                op=ALU.mult)
                            if slot == SB - 1 or w == n_win - 1:
                                flush_stage(w)

            # ---- Phase B: layer-0 edges ---------------------------------
            er0_loc = load_er_local(er0_table, er0rows, n_er0, cfg.H0, "l0")
            edge_phase("e0a", cfg.H0, cfg.D0, z0_table, er0_loc,
                       edge_in["e0a"], h_stage_a)
            edge_phase("e0b", cfg.H0, cfg.D0, z0_table, er0_loc,
                       edge_in["e0b"], h_stage_b)

            # ---- Phase C: h build + z1 shard + AllGather ----------------
            n_ht = cdiv(cfg.R0_pad, P)
            for i in range(n_ht):
                p = min(P, cfg.R0_pad - i * P)
                at = sbuf.tile([P, HD0], F32, tag="ha")
                bt = sbuf.tile([P, HD0], F32, tag="hb")
                nc.sync.dma_start(out=at[:p], in_=h_stage_a[i * P:i * P + p, :])
                nc.sync.dma_start(out=bt[:p], in_=h_stage_b[i * P:i * P + p, :])
                nc.vector.tensor_add(out=at[:p], in0=at[:p], in1=bt[:p])
                hs = sbuf.tile([P, cfg.D0], F32, tag="hs")
                nc.vector.tensor_add(out=hs[:p], in0=at[:p, 0:cfg.D0],
                                     in1=at[:p, cfg.D0:2 * cfg.D0])
                for h in range(2, cfg.H0):
                    nc.vector.tensor_add(
                        out=hs[:p], in0=hs[:p],
                        in1=at[:p, h * cfg.D0:(h + 1) * cfg.D0])
                nc.vector.tensor_add(out=hs[:p], in0=hs[:p], in1=mb2_sb[:p])
                hr = sbuf.tile([P, cfg.D0], F32, tag="hr")
                nc.scalar.activation(out=hr[:p], in_=hs[:p], func=AF.Relu,
                                     scale=1.0 / cfg.H0)
                htp = psum.tile([cfg.D0, P], F32, tag="ps_m")
                nc.tensor.transpose(out=htp[:, :p], in_=hr[:p],
                                    identity=ident[:p, :p])
                hts = sbuf.tile([cfg.D0, P], F32, tag="hts")
                nc.scalar.copy(out=hts[:, :p], in_=htp[:, :p])
                zp1 = psum.tile([P, HD1 + 2 * cfg.H1], F32, tag="ps_m")
                nc.tensor.matmul(out=zp1[:p], lhsT=hts[:, :p], rhs=W1e_sb[:],
                                 start=True, stop=True)
                z1s = sbuf.tile([P, cfg.T1], F32, tag="z1s")
                e1s = sbuf.tile([P, cfg.H1], F32, tag="e1s")
                nc.scalar.copy(out=z1s[:p], in_=zp1[:p, :cfg.T1])
                nc.scalar.copy(out=e1s[:p], in_=zp1[:p, cfg.T1:])
                nc.sync.dma_start(out=z1_shard[i * P:i * P + p, :], in_=z1s[:p])
                nc.sync.dma_start(out=er1_shard[i * P:i * P + p, :], in_=e1s[:p])

            nc.gpsimd.collective_compute(
                "AllGather", ALU.bypass,
                replica_groups=[list(range(n_cores))],
                ins=[z1_shard.opt()], outs=[z1_table.opt()])
            nc.gpsimd.collective_compute(
                "AllGather", ALU.bypass,
                replica_groups=[list(range(n_cores))],
                ins=[er1_shard.opt()], outs=[er1_table.opt()])

            # ---- Phase D: layer-1 edges ---------------------------------
            er1_loc = load_er_local(er1_table, er1rows, n_er1, cfg.H1, "l1")
            edge_phase("e1a", cfg.H1, cfg.D1, z1_table, er1_loc,
                       edge_in["e1a"], o_stage_a)
            edge_phase("e1b", cfg.H1, cfg.D1, z1_table, er1_loc,
                       edge_in["e1b"], o_stage_b)

            # ---- final combine ------------------------------------------
            n_ot = cdiv(cfg.R1_pad, P)
            for i in range(n_ot):
                p = min(P, cfg.R1_pad - i * P)
                oa = sbuf.tile([P, cfg.D1], F32, tag="oa")
                ob = sbuf.tile([P, cfg.D1], F32, tag="ob")
                nc.sync.dma_start(out=oa[:p], in_=o_stage_a[i * P:i * P + p, :])
                nc.sync.dma_start(out=ob[:p], in_=o_stage_b[i * P:i * P + p, :])
                nc.vector.tensor_add(out=oa[:p], in0=oa[:p], in1=ob[:p])
                nc.vector.tensor_add(out=oa[:p], in0=oa[:p], in1=bias1_sb[:p])
                nc.sync.dma_start(out=out_t[i * P:i * P + p, :], in_=oa[:p])

    nc.compile()
    return nc


# ----------------------------------------------------------------------------
# host driver
# ----------------------------------------------------------------------------

def make_host_inputs(cfg, feat, src0a, dst0a, src0b, dst0b, src1a, dst1a,
                     src1b, dst1b, W0, attn_l0, attn_r0, b0, W1, attn_l1,
                     attn_r1, b1):
    feat = np.asarray(feat, np.float32)
    featT = np.zeros((cfg.F_in, cfg.N0_pad), np.float32)
    featT[:, :cfg.N0] = feat.T

    ident0 = lambda i: i
    remap1 = cfg.remap1
    cpw0a, pc0a = prep_edges(np.asarray(src0a), np.asarray(dst0a), cfg.R0,
                             cfg.win, cfg.n_cores, ident0, ident0)
    cpw0b, pc0b = prep_edges(np.asarray(src0b), np.asarray(dst0b), cfg.R0,
                             cfg.win, cfg.n_cores, ident0, ident0)
    cpw1a, pc1a = prep_edges(np.asarray(src1a), np.asarray(dst1a), cfg.R1,
                             cfg.win, cfg.n_cores, remap1, remap1)
    cpw1b, pc1b = prep_edges(np.asarray(src1b), np.asarray(dst1b), cfg.R1,
                             cfg.win, cfg.n_cores, remap1, remap1)

    shared = dict(
        featT=featT,
        W0=np.asarray(W0, np.float32),
        Aler0=np.concatenate([block_diag_attn(np.asarray(attn_l0, np.float32)),
                              block_diag_attn(np.asarray(attn_r0, np.float32))],
                             axis=1),
        b0r=np.asarray(b0, np.float32).reshape(cfg.H0, cfg.D0),
        W1=np.asarray(W1, np.float32),
        Aler1=np.concatenate([block_diag_attn(np.asarray(attn_l1, np.float32)),
                              block_diag_attn(np.asarray(attn_r1, np.float32))],
                             axis=1),
        b1r=np.asarray(b1, np.float32).reshape(1, cfg.D1),
        iota=np.tile(np.arange(cfg.win, dtype=np.float32), (P, 1)),
        ones4=np.ones((cfg.H0, 1), np.float32),
        twos=np.full((1, P), 2.0, np.float32),
    )
    n_er0 = cdiv(cfg.R0_pad, P)
    n_er1 = cdiv(cfg.R1_pad, P)
    pp = np.arange(P)[:, None]
    in_maps = []
    for c in range(cfg.n_cores):
        m = dict(shared)
        m["er0rows"] = (c * cfg.R0 + np.arange(n_er0)[None, :] * P
                        + pp).astype(np.int32)
        d1 = c * cfg.R1 + np.arange(n_er1)[None, :] * P + pp
        m["er1rows"] = cfg.remap1(np.minimum(d1, cfg.N1 - 1)).astype(np.int32)
        for name, pc in (("e0a", pc0a), ("e0b", pc0b), ("e1a", pc1a),
                         ("e1b", pc1b)):
            m[f"{name}_src"] = pc[c]["srcT"]
            m[f"{name}_col"] = pc[c]["colT"]
        in_maps.append(m)
    return in_maps, (cpw0a, cpw0b, cpw1a, cpw1b)


def assemble_output(cfg, results):
    outs = [results[c]["out"][:cfg.R1, :] for c in range(cfg.n_cores)]
    out = np.concatenate(outs, axis=0)
    return out.reshape(cfg.N2, cfg.H1, cfg.D1).astype(np.float32)


_CACHED = {}


def kernel(**inputs):
    cfg = Cfg()
    in_maps, cpws = make_host_inputs(cfg, **inputs)
    key = tuple(tuple(c) for c in cpws)
    if key not in _CACHED:
        _CACHED[key] = build_program(cfg, *cpws)
    nc = _CACHED[key]
    res = run_bass_kernel_spmd(nc, in_maps, list(range(cfg.n_cores)))
    return assemble_output(cfg, res.results)
